# revision 24
# baseline (speedup 1.0000x reference)
"""Octahedral SHT on 8 NeuronCores (Bass/Tile) — v6.

Dataflow per core:
  phase 1 (per ring-pair group g): psum gre/gim [m=128, 256=(ns,bev)]
  accumulated over the group's chunk slots; 4 copies -> gsl span tile
  [m, (ns,h,bev)] fp16; span write to ring-major gdram rows 2g+ns
  (col = m*256 + h*128 + bev).
  bounce reads: gbuf [128, 8192] holds G q-stacked: row 32q + ring keeps
  m in [32q, 32q+32), col (m%32)*256 + h*128 + bev. Filled by 8 PLAIN 2D
  reads (per q, two ring-halves) — exact-shape DMAs, many SBUF rows per
  read so all DMA engines engage.
  phase 2 (per m, h): q = m//32: matmul(out[bev, L],
  lhsT=gbuf[32q:32q+24, (m%32)*256+h*128:+128],
  rhs=pw4[32q:32q+24, OFF[m]:+L], tile base 32q), L = 128-m (exact
  l >= m triangle); greedy-packed psum banks, scalar/vector evacuation,
  osb staging of 4 banks per output DMA. Host sums the 8 partials.

Performance rules this version is built around (from the TRN2 cost model
and traces):
  * each dma_start costs ~0.65us DGE on its queue and a queue has only
    ~8-10 semaphores — more than that inserts engine-blocking reuse
    guards. FEW, BIG DMAs, <=10 per queue (sync / scalar / gpsimd).
  * DMA engine assignment goes by SBUF row: a transfer touching few SBUF
    partitions runs at ~15GB/s per row. Every DMA here spans >=18 rows.
  * matmul operands need base partition in {0,32,64,96} (k<=32) and
    LDWEIGHTS cannot stride partitions -> q-outer layout + pw replicated
    per 32-row block (host-built pw4).
  * the PE p-state ramp needs 3us of continuous execution to hit 2.4GHz
    and resets on stalls — filler matmuls bridge the phase-1 -> phase-2
    boundary.
  * xe and ee interleave per-slot in one xee tensor so one DMA stream
    feeds both sides of the phase-1 matmuls in consumption order.

Sharding: 204 north DFT chunks -> 8 cores x 27 slots as 12 ring-pair
groups per core, sizes [4,3,3,3,3,2,2,2,2,1,1,1]; each slot carries the
mirrored south ring's chunk (same E). Host sums the 8 partial outputs.
"""
import numpy as np

NLAT, LMAX, MMAX = 192, 128, 128
B, V = 2, 64
BF = B * V
NCORES = 8
CHUNK = 128
GSIZES = [4, 3, 3, 3, 3, 2, 2, 2, 2, 1, 1, 1]
NG = len(GSIZES)                  # 12 ring-pair groups per core
NSLOT = sum(GSIZES)               # 27
NROWS = 2 * NG                    # 24 rings per core
GOFF = np.concatenate([[0], np.cumsum(GSIZES)]).astype(np.int64)
MAX_NLON = 400
NPTS = 40320

# exact triangle: m-th block has L = 128-m columns (l = m..127)
LLEN = [LMAX - m for m in range(MMAX)]
OFF = np.concatenate([[0], np.cumsum(LLEN)]).astype(np.int64)
PWW = int(OFF[-1])                # 8256
OUTW = 2 * PWW                    # 16512 (re & im per m)

# xee load batches (slot_start, n_slots), group-aligned; alternate
# sync/gpsimd queues in order
XEEB = [(0, 1), (1, 1), (2, 2), (4, 3), (7, 6), (13, 5), (18, 4), (22, 5)]
assert sum(n for _, n in XEEB) == NSLOT

# bounce write spans (first_group, n_groups); last span is one group so
# the final write+read chain gating phase 2 is short
SPANS = [(0, 4), (4, 4), (8, 2), (10, 2)]

N_FILLER = 30


def _octa_nlon():
    half = NLAT // 2
    north = np.array([4 * (i + 1) + 16 for i in range(half)], dtype=np.int64)
    return np.concatenate([north, north[::-1]])


def _assign_groups():
    nlon = _octa_nlon()
    nch = np.ceil(nlon[:96] / CHUNK).astype(int)
    cls = {c: sorted(np.where(nch == c)[0].tolist()) for c in (1, 2, 3, 4)}
    assert [len(cls[c]) for c in (1, 2, 3, 4)] == [28, 32, 32, 4]
    c1, c2, c3, c4 = cls[1][:], cls[2][:], cls[3][:], cls[4][:]
    cores = []
    for c in range(NCORES):
        g4 = c4.pop() if c < 4 else c2.pop()
        g3s = [c3.pop() for _ in range(4)]
        g2s = [c2.pop() for _ in range(4)] if c < 4 else \
              [c2.pop() for _ in range(3)] + [c1.pop()]
        g1s = [c1.pop() for _ in range(3)]
        cores.append([g4] + g3s + g2s + g1s)
    assert not c1 and not c2 and not c3 and not c4
    return cores, nlon


def _build_core(rings, nlon, offs, x, E_re, E_im, PwT):
    """xee: [128 j, slot, (xe 2*BF | ee 2*MMAX)] fp16; pw4: [128, PWW]."""
    xee = np.zeros((CHUNK, NSLOT, 512), np.float16)
    pw4 = np.zeros((96, PWW), np.float16)
    for g in range(NG):
        r = rings[g]
        rs = NLAT - 1 - r
        nl = int(nlon[r])
        for t in range(GSIZES[g]):
            j0 = t * CHUNK
            if j0 >= nl:
                continue
            s = int(GOFF[g]) + t
            jlen = min(CHUNK, nl - j0)
            xee[:jlen, s, 0:BF] = x[:, offs[r] + j0: offs[r] + j0 + jlen].T
            xee[:jlen, s, BF:2 * BF] = \
                x[:, offs[rs] + j0: offs[rs] + j0 + jlen].T
            elen = min(CHUNK, MAX_NLON - j0)
            xee[:elen, s, 256:256 + MMAX] = E_re[r, j0:j0 + elen, :]
            xee[:elen, s, 256 + MMAX:] = E_im[r, j0:j0 + elen, :]
        for ns, rr in ((0, r), (1, rs)):
            row = 2 * g + ns
            for m in range(MMAX):
                pw4[row, int(OFF[m]):int(OFF[m + 1])] = PwT[m, rr, m:]
    pw4[32:56] = pw4[0:24]
    pw4[64:88] = pw4[0:24]
    return np.ascontiguousarray(xee.reshape(CHUNK, NSLOT * 512)), pw4


def _build_bass():
    import concourse.mybir as mybir
    from concourse import bacc, tile

    dt = mybir.dt
    nc = bacc.Bacc()

    xee_d = nc.dram_tensor("xee", [CHUNK, NSLOT * 512], dt.float16,
                           kind="ExternalInput")
    pw_d = nc.dram_tensor("pw4", [96, PWW], dt.float16,
                          kind="ExternalInput")
    outp_d = nc.dram_tensor("outp", [128, OUTW], dt.float16,
                            kind="ExternalOutput")
    # ring-major bounce: row 2g+ns, col m*256 + h*128 + bev
    gdram = nc.dram_tensor("gdram", [NROWS, MMAX * 256], dt.float16)

    with tile.TileContext(nc) as tc:
        with (
            tc.tile_pool(name="inp", bufs=1) as in_pool,
            tc.tile_pool(name="gsl", bufs=3) as gsl_pool,
            tc.tile_pool(name="osb", bufs=4) as osb_pool,
            tc.tile_pool(name="ps1a", bufs=2, space="PSUM") as ps1a,
            tc.tile_pool(name="ps1b", bufs=2, space="PSUM") as ps1b,
            tc.tile_pool(name="ps2", bufs=4, space="PSUM") as ps2,
        ):
            xee = in_pool.tile([CHUNK, NSLOT * 512], dt.float16, tag="xee")
            # pw replicated to base partitions 0/32/64 (matmul base rule);
            # quarter 3 shares base 64 via a second gbuf column range
            pw4 = in_pool.tile([96, PWW], dt.float16, tag="pw4")
            gbuf = in_pool.tile([96, 2 * 32 * 256], dt.float16, tag="gbuf")

            # --- input loads: alternate sync/gpsimd (consumption order
            # within each queue); replicated pw4 in two loads on gpsimd ---
            for i, (s0, n) in enumerate(XEEB):
                c0, c1 = s0 * 512, (s0 + n) * 512
                q = (nc.sync, nc.gpsimd)[i % 2]
                q.dma_start(out=xee[:, c0:c1], in_=xee_d[:, c0:c1])
            nc.sync.dma_start(out=pw4[0:48, :], in_=pw_d[0:48, :])
            nc.gpsimd.dma_start(out=pw4[48:96, :], in_=pw_d[48:96, :])

            def xs(s):          # xe part of slot s (rhs)
                return xee[:, s * 512:s * 512 + 256]

            def es(s, h):       # ee part of slot s, h=0 re / h=1 im (lhsT)
                o = s * 512 + 256 + h * 128
                return xee[:, o:o + 128]

            def cp1(idx, out, in_):
                if idx % 2 == 0:
                    nc.scalar.copy(out, in_)
                else:
                    nc.vector.tensor_copy(out, in_)

            span_of = {}
            for si, (g0, ng) in enumerate(SPANS):
                for g in range(g0, g0 + ng):
                    span_of[g] = si

            # --- phase 1: groups; span writes on scalar; q-stacked plain
            # reads on sync/gpsimd after the right span ---
            ci = 0
            gsl_cur = None
            for g in range(NG):
                si = span_of[g]
                g0, ng = SPANS[si]
                sz = GSIZES[g]
                gre = ps1a.tile([MMAX, 512], dt.float32, tag="gre")
                gim = ps1b.tile([MMAX, 512], dt.float32, tag="gim")
                for t in range(sz):
                    s = int(GOFF[g]) + t
                    st, sp = (t == 0), (t == sz - 1)
                    nc.tensor.matmul(gre[:, 0:256], es(s, 0), xs(s),
                                     start=st, stop=sp)
                    nc.tensor.matmul(gim[:, 0:256], es(s, 1), xs(s),
                                     start=st, stop=sp)
                if g == g0:
                    gsl_cur = gsl_pool.tile([MMAX, 512 * ng], dt.float16,
                                            tag=f"gsl{ng}", name="gsl")
                go = 512 * (g - g0)
                cp1(ci + 0, gsl_cur[:, go + 0:go + 128], gre[:, 0:128])
                cp1(ci + 1, gsl_cur[:, go + 128:go + 256], gim[:, 0:128])
                cp1(ci + 2, gsl_cur[:, go + 256:go + 384], gre[:, 128:256])
                cp1(ci + 3, gsl_cur[:, go + 384:go + 512], gim[:, 128:256])
                ci += 4
                if g == g0 + ng - 1:
                    # span write into ring-major rows (src 128 SBUF rows,
                    # 512B descriptors); sync has no compute duties
                    wdst = gdram[2 * g0:2 * (g0 + ng)].rearrange(
                        "k (m e) -> m k e", m=MMAX)
                    wsrc = gsl_cur[:, :].rearrange(
                        "m (k e) -> m k e", k=2 * ng)
                    wq = nc.sync if si % 2 == 0 else nc.gpsimd
                    wq.dma_start(out=wdst, in_=wsrc)
                    # q-stacked plain full reads after the last span; q0
                    # first — phase 2's first m-block only needs q0
                    if si == len(SPANS) - 1:
                        rqs = (nc.gpsimd, nc.scalar, nc.sync, nc.scalar)
                        for q4 in range(4):
                            rb = 32 * q4 if q4 < 3 else 64
                            cb = 0 if q4 < 3 else 8192
                            rqs[q4].dma_start(
                                out=gbuf[rb:rb + NROWS, cb:cb + 8192],
                                in_=gdram[:, q4 * 8192:(q4 + 1) * 8192])

            # --- fillers: keep the PE p-state ramp alive across the
            # phase-1 -> phase-2 boundary (psum scratch, never read) ---
            for f in range(N_FILLER):
                pool, tg = (ps1a, "gre") if f % 2 == 0 else (ps1b, "gim")
                fil = pool.tile([MMAX, 512], dt.float32, tag=tg, name="fil")
                nc.tensor.matmul(fil[:, 0:512], es(0, 0), xee[:, 0:512],
                                 start=True, stop=True)

            # --- phase 2 ---
            oq = (nc.sync, nc.gpsimd, nc.sync, nc.gpsimd, nc.scalar)
            state = {"osb": None, "osb_fill": 0, "osb_base": 0, "banks": 0,
                     "po": None, "po_fill": 0, "oi": 0, "col": 0, "ci": ci}

            def cp2(idx, out, in_):
                if idx % 2 == 0:
                    nc.vector.tensor_copy(out, in_)
                else:
                    nc.scalar.copy(out, in_)

            def flush_bank():
                if state["po"] is None or state["po_fill"] == 0:
                    return
                f = state["osb_fill"]
                cp2(state["ci"], state["osb"][:, f:f + state["po_fill"]],
                    state["po"][:, 0:state["po_fill"]])
                state["ci"] += 1
                state["osb_fill"] = f + state["po_fill"]
                state["banks"] += 1
                state["po"] = None
                state["po_fill"] = 0

            def flush_osb():
                if state["osb"] is None or state["osb_fill"] == 0:
                    return
                oq[state["oi"] % 5].dma_start(
                    out=outp_d[:, state["osb_base"]:
                               state["osb_base"] + state["osb_fill"]],
                    in_=state["osb"][:, 0:state["osb_fill"]])
                state["oi"] += 1
                state["osb"] = None
                state["osb_fill"] = 0
                state["banks"] = 0

            for m in range(MMAX):
                L = LLEN[m]
                q4, ml = m // 32, m % 32
                rb = 32 * q4 if q4 < 3 else 64
                cb = 0 if q4 < 3 else 8192
                rhs = pw4[rb:rb + NROWS, int(OFF[m]):int(OFF[m]) + L]
                if ml == 0 and m > 0:
                    # matmuls with different base partitions cannot share a
                    # psum bank — break banks at quarter boundaries
                    flush_bank()
                    if state["banks"] == 8:
                        flush_osb()
                for h in range(2):
                    if state["po"] is not None and state["po_fill"] + L > 512:
                        flush_bank()
                        if state["banks"] == 8:
                            flush_osb()
                    if state["po"] is None:
                        if state["osb"] is None:
                            state["osb"] = osb_pool.tile(
                                [128, 4096], dt.float16, tag="osb",
                                name="osb")
                            state["osb_base"] = state["col"]
                        bi = state.get("bi", 0)
                        pool, tg = ((ps2, "po"), (ps2, "po"), (ps2, "po"),
                                    (ps2, "po"), (ps1a, "gre"),
                                    (ps1b, "gim"))[bi % 6]
                        state["bi"] = bi + 1
                        state["po"] = pool.tile([128, 512], dt.float32,
                                                tag=tg, name="po")
                    c0 = cb + ml * 256 + h * 128
                    lhsT = gbuf[rb:rb + NROWS, c0:c0 + 128]
                    nc.tensor.matmul(
                        state["po"][:, state["po_fill"]:
                                    state["po_fill"] + L],
                        lhsT, rhs, start=True, stop=True)
                    state["po_fill"] += L
                    state["col"] += L
            flush_bank()
            flush_osb()

    nc.compile()
    return nc


_CACHE = {}


def _get_compiled():
    if "nc" not in _CACHE:
        _CACHE["nc"] = _build_bass()
    return _CACHE["nc"]


def kernel(data, Pw, E_re, E_im, pad_idx):
    from concourse import bass_utils

    data = np.asarray(data)
    Pw = np.asarray(Pw, dtype=np.float32)
    E_re = np.asarray(E_re, dtype=np.float32)
    E_im = np.asarray(E_im, dtype=np.float32)

    cores, nlon = _assign_groups()
    offs = np.concatenate([[0], np.cumsum(nlon)[:-1]])
    x = np.ascontiguousarray(
        np.transpose(data, (0, 1, 3, 2)).reshape(BF, NPTS).astype(np.float32))
    PwT = np.ascontiguousarray(np.transpose(Pw, (1, 2, 0)))  # [m, n, l]

    in_maps = []
    for c in range(NCORES):
        xee, pw4 = _build_core(cores[c], nlon, offs, x, E_re, E_im, PwT)
        in_maps.append({"xee": xee, "pw4": pw4})

    nc = _get_compiled()
    res = bass_utils.run_bass_kernel_spmd(nc, in_maps, list(range(NCORES)))
    _CACHE["last_results"] = res

    total = np.zeros((BF, OUTW), np.float64)
    for r in res.results:
        total += r["outp"].astype(np.float64)
    total = total.astype(np.float32)

    cc = np.zeros((LMAX, MMAX, BF), np.complex64)
    for m in range(MMAX):
        L = LLEN[m]
        o = 2 * int(OFF[m])
        re = total[:, o:o + L]
        im = total[:, o + L:o + 2 * L]
        cc[m:, m, :] = (re + 1j * im).T
    cc = cc.reshape(LMAX, MMAX, B, V)
    out = np.transpose(cc, (2, 0, 1, 3))[:, None]
    return out.astype(np.complex64)


# revision 25
# speedup vs baseline: 1.0499x; 1.0499x over previous
"""Octahedral SHT on 8 NeuronCores (Bass/Tile) — v6.

Dataflow per core:
  phase 1 (per ring-pair group g): psum gre/gim [m=128, 256=(ns,bev)]
  accumulated over the group's chunk slots; 4 copies -> gsl span tile
  [m, (ns,h,bev)] fp16; span write to ring-major gdram rows 2g+ns
  (col = m*256 + h*128 + bev).
  bounce reads: gbuf [128, 8192] holds G q-stacked: row 32q + ring keeps
  m in [32q, 32q+32), col (m%32)*256 + h*128 + bev. Filled by 8 PLAIN 2D
  reads (per q, two ring-halves) — exact-shape DMAs, many SBUF rows per
  read so all DMA engines engage.
  phase 2 (per m, h): q = m//32: matmul(out[bev, L],
  lhsT=gbuf[32q:32q+24, (m%32)*256+h*128:+128],
  rhs=pw4[32q:32q+24, OFF[m]:+L], tile base 32q), L = 128-m (exact
  l >= m triangle); greedy-packed psum banks, scalar/vector evacuation,
  osb staging of 4 banks per output DMA. Host sums the 8 partials.

Performance rules this version is built around (from the TRN2 cost model
and traces):
  * each dma_start costs ~0.65us DGE on its queue and a queue has only
    ~8-10 semaphores — more than that inserts engine-blocking reuse
    guards. FEW, BIG DMAs, <=10 per queue (sync / scalar / gpsimd).
  * DMA engine assignment goes by SBUF row: a transfer touching few SBUF
    partitions runs at ~15GB/s per row. Every DMA here spans >=18 rows.
  * matmul operands need base partition in {0,32,64,96} (k<=32) and
    LDWEIGHTS cannot stride partitions -> q-outer layout + pw replicated
    per 32-row block (host-built pw4).
  * the PE p-state ramp needs 3us of continuous execution to hit 2.4GHz
    and resets on stalls — filler matmuls bridge the phase-1 -> phase-2
    boundary.
  * xe and ee interleave per-slot in one xee tensor so one DMA stream
    feeds both sides of the phase-1 matmuls in consumption order.

Sharding: 204 north DFT chunks -> 8 cores x 27 slots as 12 ring-pair
groups per core, sizes [4,3,3,3,3,2,2,2,2,1,1,1]; each slot carries the
mirrored south ring's chunk (same E). Host sums the 8 partial outputs.
"""
import numpy as np

NLAT, LMAX, MMAX = 192, 128, 128
B, V = 2, 64
BF = B * V
NCORES = 8
CHUNK = 128
GSIZES = [4, 3, 3, 3, 3, 2, 2, 2, 2, 1, 1, 1]
NG = len(GSIZES)                  # 12 ring-pair groups per core
NSLOT = sum(GSIZES)               # 27
NROWS = 2 * NG                    # 24 rings per core
GOFF = np.concatenate([[0], np.cumsum(GSIZES)]).astype(np.int64)
MAX_NLON = 400
NPTS = 40320

# exact triangle: m-th block has L = 128-m columns (l = m..127)
LLEN = [LMAX - m for m in range(MMAX)]
OFF = np.concatenate([[0], np.cumsum(LLEN)]).astype(np.int64)
PWW = int(OFF[-1])                # 8256
OUTW = 2 * PWW                    # 16512 (re & im per m)

# xee load batches (slot_start, n_slots), group-aligned; alternate
# sync/gpsimd queues in order
XEEB = [(0, 1), (1, 1), (2, 2), (4, 3), (7, 6), (13, 5), (18, 4), (22, 5)]
assert sum(n for _, n in XEEB) == NSLOT

# bounce write spans (first_group, n_groups); last span is one group so
# the final write+read chain gating phase 2 is short
SPANS = [(0, 4), (4, 4), (8, 2), (10, 2)]

N_FILLER = 30


def _octa_nlon():
    half = NLAT // 2
    north = np.array([4 * (i + 1) + 16 for i in range(half)], dtype=np.int64)
    return np.concatenate([north, north[::-1]])


def _assign_groups():
    nlon = _octa_nlon()
    nch = np.ceil(nlon[:96] / CHUNK).astype(int)
    cls = {c: sorted(np.where(nch == c)[0].tolist()) for c in (1, 2, 3, 4)}
    assert [len(cls[c]) for c in (1, 2, 3, 4)] == [28, 32, 32, 4]
    c1, c2, c3, c4 = cls[1][:], cls[2][:], cls[3][:], cls[4][:]
    cores = []
    for c in range(NCORES):
        g4 = c4.pop() if c < 4 else c2.pop()
        g3s = [c3.pop() for _ in range(4)]
        g2s = [c2.pop() for _ in range(4)] if c < 4 else \
              [c2.pop() for _ in range(3)] + [c1.pop()]
        g1s = [c1.pop() for _ in range(3)]
        cores.append([g4] + g3s + g2s + g1s)
    assert not c1 and not c2 and not c3 and not c4
    return cores, nlon


def _build_core(rings, nlon, offs, x, E_re, E_im, PwT):
    """xee: [128 j, slot, (xe 2*BF | ee 2*MMAX)] fp16; pw4: [128, PWW]."""
    xee = np.zeros((CHUNK, NSLOT, 512), np.float16)
    pw = np.zeros((NROWS, PWW), np.float16)
    for g in range(NG):
        r = rings[g]
        rs = NLAT - 1 - r
        nl = int(nlon[r])
        for t in range(GSIZES[g]):
            j0 = t * CHUNK
            if j0 >= nl:
                continue
            s = int(GOFF[g]) + t
            jlen = min(CHUNK, nl - j0)
            xee[:jlen, s, 0:BF] = x[:, offs[r] + j0: offs[r] + j0 + jlen].T
            xee[:jlen, s, BF:2 * BF] = \
                x[:, offs[rs] + j0: offs[rs] + j0 + jlen].T
            elen = min(CHUNK, MAX_NLON - j0)
            xee[:elen, s, 256:256 + MMAX] = E_re[r, j0:j0 + elen, :]
            xee[:elen, s, 256 + MMAX:] = E_im[r, j0:j0 + elen, :]
        for ns, rr in ((0, r), (1, rs)):
            row = 2 * g + ns
            for m in range(MMAX):
                pw[row, int(OFF[m]):int(OFF[m + 1])] = PwT[m, rr, m:]
    return np.ascontiguousarray(xee.reshape(CHUNK, NSLOT * 512)), pw


def _build_bass():
    import concourse.mybir as mybir
    from concourse import bacc, tile

    dt = mybir.dt
    nc = bacc.Bacc()

    xee_d = nc.dram_tensor("xee", [CHUNK, NSLOT * 512], dt.float16,
                           kind="ExternalInput")
    pw_d = nc.dram_tensor("pw", [NROWS, PWW], dt.float16,
                          kind="ExternalInput")
    outp_d = nc.dram_tensor("outp", [128, OUTW], dt.float16,
                            kind="ExternalOutput")
    # ring-major bounce: row 2g+ns, col m*256 + h*128 + bev
    gdram = nc.dram_tensor("gdram", [NROWS, MMAX * 256], dt.float16)

    with tile.TileContext(nc) as tc:
        with (
            tc.tile_pool(name="inp", bufs=1) as in_pool,
            tc.tile_pool(name="gsl", bufs=3) as gsl_pool,
            tc.tile_pool(name="osb", bufs=4) as osb_pool,
            tc.tile_pool(name="ps1a", bufs=2, space="PSUM") as ps1a,
            tc.tile_pool(name="ps1b", bufs=2, space="PSUM") as ps1b,
            tc.tile_pool(name="ps2", bufs=4, space="PSUM") as ps2,
        ):
            xee = in_pool.tile([CHUNK, NSLOT * 512], dt.float16, tag="xee")
            # pw replicated to base partitions 0/32/64 (matmul base rule);
            # quarter 3 shares base 64 via a second gbuf column range
            pw4 = in_pool.tile([96, PWW], dt.float16, tag="pw4")
            gbuf = in_pool.tile([96, 2 * 32 * 256], dt.float16, tag="gbuf")

            # --- input loads: alternate sync/gpsimd (consumption order
            # within each queue); replicated pw4 in two loads on gpsimd ---
            for i, (s0, n) in enumerate(XEEB):
                c0, c1 = s0 * 512, (s0 + n) * 512
                q = (nc.sync, nc.gpsimd)[i % 2]
                q.dma_start(out=xee[:, c0:c1], in_=xee_d[:, c0:c1])
            nc.gpsimd.dma_start(out=pw4[0:NROWS, :], in_=pw_d[:])

            def xs(s):          # xe part of slot s (rhs)
                return xee[:, s * 512:s * 512 + 256]

            def es(s, h):       # ee part of slot s, h=0 re / h=1 im (lhsT)
                o = s * 512 + 256 + h * 128
                return xee[:, o:o + 128]

            def cp1(idx, out, in_):
                if idx % 2 == 0:
                    nc.scalar.copy(out, in_)
                else:
                    nc.vector.tensor_copy(out, in_)

            span_of = {}
            for si, (g0, ng) in enumerate(SPANS):
                for g in range(g0, g0 + ng):
                    span_of[g] = si

            # --- phase 1: groups; span writes on scalar; q-stacked plain
            # reads on sync/gpsimd after the right span ---
            ci = 0
            gsl_cur = None
            for g in range(NG):
                si = span_of[g]
                g0, ng = SPANS[si]
                sz = GSIZES[g]
                gre = ps1a.tile([MMAX, 512], dt.float32, tag="gre")
                gim = ps1b.tile([MMAX, 512], dt.float32, tag="gim")
                for t in range(sz):
                    s = int(GOFF[g]) + t
                    st, sp = (t == 0), (t == sz - 1)
                    nc.tensor.matmul(gre[:, 0:256], es(s, 0), xs(s),
                                     start=st, stop=sp)
                    nc.tensor.matmul(gim[:, 0:256], es(s, 1), xs(s),
                                     start=st, stop=sp)
                if g == g0:
                    gsl_cur = gsl_pool.tile([MMAX, 512 * ng], dt.float16,
                                            tag=f"gsl{ng}", name="gsl")
                go = 512 * (g - g0)
                cp1(ci + 0, gsl_cur[:, go + 0:go + 128], gre[:, 0:128])
                cp1(ci + 1, gsl_cur[:, go + 128:go + 256], gim[:, 0:128])
                cp1(ci + 2, gsl_cur[:, go + 256:go + 384], gre[:, 128:256])
                cp1(ci + 3, gsl_cur[:, go + 384:go + 512], gim[:, 128:256])
                ci += 4
                if g == g0 + ng - 1:
                    # span write into ring-major rows (src 128 SBUF rows,
                    # 512B descriptors); sync has no compute duties
                    wdst = gdram[2 * g0:2 * (g0 + ng)].rearrange(
                        "k (m e) -> m k e", m=MMAX)
                    wsrc = gsl_cur[:, :].rearrange(
                        "m (k e) -> m k e", k=2 * ng)
                    wq = nc.sync if si % 2 == 0 else nc.gpsimd
                    wq.dma_start(out=wdst, in_=wsrc)
                    # q-stacked plain full reads after the last span; q0
                    # first — phase 2's first m-block only needs q0
                    if si == len(SPANS) - 1:
                        rqs = (nc.gpsimd, nc.scalar, nc.sync, nc.scalar)
                        for q4 in range(4):
                            rb = 32 * q4 if q4 < 3 else 64
                            cb = 0 if q4 < 3 else 8192
                            rqs[q4].dma_start(
                                out=gbuf[rb:rb + NROWS, cb:cb + 8192],
                                in_=gdram[:, q4 * 8192:(q4 + 1) * 8192])
                        # pw replicas to bases 32/64 — needed once phase 2
                        # reaches m=32, well after these queue slots
                        nc.gpsimd.dma_start(out=pw4[32:32 + NROWS, :],
                                            in_=pw4[0:NROWS, :])
                        nc.gpsimd.dma_start(out=pw4[64:64 + NROWS, :],
                                            in_=pw4[0:NROWS, :])

            # --- fillers: keep the PE p-state ramp alive across the
            # phase-1 -> phase-2 boundary (psum scratch, never read) ---
            for f in range(N_FILLER):
                pool, tg = (ps1a, "gre") if f % 2 == 0 else (ps1b, "gim")
                fil = pool.tile([MMAX, 512], dt.float32, tag=tg, name="fil")
                nc.tensor.matmul(fil[:, 0:512], es(0, 0), xee[:, 0:512],
                                 start=True, stop=True)

            # --- phase 2 ---
            oq = (nc.sync, nc.scalar, nc.sync, nc.scalar, nc.sync)
            state = {"osb": None, "osb_fill": 0, "osb_base": 0, "banks": 0,
                     "po": None, "po_fill": 0, "oi": 0, "col": 0, "ci": ci}

            def cp2(idx, out, in_):
                if idx % 2 == 0:
                    nc.vector.tensor_copy(out, in_)
                else:
                    nc.scalar.copy(out, in_)

            def flush_bank():
                if state["po"] is None or state["po_fill"] == 0:
                    return
                f = state["osb_fill"]
                cp2(state["ci"], state["osb"][:, f:f + state["po_fill"]],
                    state["po"][:, 0:state["po_fill"]])
                state["ci"] += 1
                state["osb_fill"] = f + state["po_fill"]
                state["banks"] += 1
                state["po"] = None
                state["po_fill"] = 0

            def flush_osb():
                if state["osb"] is None or state["osb_fill"] == 0:
                    return
                oq[state["oi"] % 5].dma_start(
                    out=outp_d[:, state["osb_base"]:
                               state["osb_base"] + state["osb_fill"]],
                    in_=state["osb"][:, 0:state["osb_fill"]])
                state["oi"] += 1
                state["osb"] = None
                state["osb_fill"] = 0
                state["banks"] = 0

            for m in range(MMAX):
                L = LLEN[m]
                q4, ml = m // 32, m % 32
                rb = 32 * q4 if q4 < 3 else 64
                cb = 0 if q4 < 3 else 8192
                rhs = pw4[rb:rb + NROWS, int(OFF[m]):int(OFF[m]) + L]
                if ml == 0 and m > 0:
                    # matmuls with different base partitions cannot share a
                    # psum bank — break banks at quarter boundaries
                    flush_bank()
                    if state["banks"] == 8:
                        flush_osb()
                for h in range(2):
                    if state["po"] is not None and state["po_fill"] + L > 512:
                        flush_bank()
                        if state["banks"] == 8:
                            flush_osb()
                    if state["po"] is None:
                        if state["osb"] is None:
                            state["osb"] = osb_pool.tile(
                                [128, 4096], dt.float16, tag="osb",
                                name="osb")
                            state["osb_base"] = state["col"]
                        bi = state.get("bi", 0)
                        pool, tg = ((ps2, "po"), (ps2, "po"), (ps2, "po"),
                                    (ps2, "po"), (ps1a, "gre"),
                                    (ps1b, "gim"))[bi % 6]
                        state["bi"] = bi + 1
                        state["po"] = pool.tile([128, 512], dt.float32,
                                                tag=tg, name="po")
                    c0 = cb + ml * 256 + h * 128
                    lhsT = gbuf[rb:rb + NROWS, c0:c0 + 128]
                    nc.tensor.matmul(
                        state["po"][:, state["po_fill"]:
                                    state["po_fill"] + L],
                        lhsT, rhs, start=True, stop=True)
                    state["po_fill"] += L
                    state["col"] += L
            flush_bank()
            flush_osb()

    nc.compile()
    return nc


_CACHE = {}


def _get_compiled():
    if "nc" not in _CACHE:
        _CACHE["nc"] = _build_bass()
    return _CACHE["nc"]


def kernel(data, Pw, E_re, E_im, pad_idx):
    from concourse import bass_utils

    data = np.asarray(data)
    Pw = np.asarray(Pw, dtype=np.float32)
    E_re = np.asarray(E_re, dtype=np.float32)
    E_im = np.asarray(E_im, dtype=np.float32)

    cores, nlon = _assign_groups()
    offs = np.concatenate([[0], np.cumsum(nlon)[:-1]])
    x = np.ascontiguousarray(
        np.transpose(data, (0, 1, 3, 2)).reshape(BF, NPTS).astype(np.float32))
    PwT = np.ascontiguousarray(np.transpose(Pw, (1, 2, 0)))  # [m, n, l]

    in_maps = []
    for c in range(NCORES):
        xee, pw = _build_core(cores[c], nlon, offs, x, E_re, E_im, PwT)
        in_maps.append({"xee": xee, "pw": pw})

    nc = _get_compiled()
    res = bass_utils.run_bass_kernel_spmd(nc, in_maps, list(range(NCORES)))
    _CACHE["last_results"] = res

    total = np.zeros((BF, OUTW), np.float64)
    for r in res.results:
        total += r["outp"].astype(np.float64)
    total = total.astype(np.float32)

    cc = np.zeros((LMAX, MMAX, BF), np.complex64)
    for m in range(MMAX):
        L = LLEN[m]
        o = 2 * int(OFF[m])
        re = total[:, o:o + L]
        im = total[:, o + L:o + 2 * L]
        cc[m:, m, :] = (re + 1j * im).T
    cc = cc.reshape(LMAX, MMAX, B, V)
    out = np.transpose(cc, (2, 0, 1, 3))[:, None]
    return out.astype(np.complex64)


# revision 26
# speedup vs baseline: 1.0891x; 1.0373x over previous
"""Octahedral SHT on 8 NeuronCores (Bass/Tile) — v6.

Dataflow per core:
  phase 1 (per ring-pair group g): psum gre/gim [m=128, 256=(ns,bev)]
  accumulated over the group's chunk slots; 4 copies -> gsl span tile
  [m, (ns,h,bev)] fp16; span write to ring-major gdram rows 2g+ns
  (col = m*256 + h*128 + bev).
  bounce reads: gbuf [128, 8192] holds G q-stacked: row 32q + ring keeps
  m in [32q, 32q+32), col (m%32)*256 + h*128 + bev. Filled by 8 PLAIN 2D
  reads (per q, two ring-halves) — exact-shape DMAs, many SBUF rows per
  read so all DMA engines engage.
  phase 2 (per m, h): q = m//32: matmul(out[bev, L],
  lhsT=gbuf[32q:32q+24, (m%32)*256+h*128:+128],
  rhs=pw4[32q:32q+24, OFF[m]:+L], tile base 32q), L = 128-m (exact
  l >= m triangle); greedy-packed psum banks, scalar/vector evacuation,
  osb staging of 4 banks per output DMA. Host sums the 8 partials.

Performance rules this version is built around (from the TRN2 cost model
and traces):
  * each dma_start costs ~0.65us DGE on its queue and a queue has only
    ~8-10 semaphores — more than that inserts engine-blocking reuse
    guards. FEW, BIG DMAs, <=10 per queue (sync / scalar / gpsimd).
  * DMA engine assignment goes by SBUF row: a transfer touching few SBUF
    partitions runs at ~15GB/s per row. Every DMA here spans >=18 rows.
  * matmul operands need base partition in {0,32,64,96} (k<=32) and
    LDWEIGHTS cannot stride partitions -> q-outer layout + pw replicated
    per 32-row block (host-built pw4).
  * the PE p-state ramp needs 3us of continuous execution to hit 2.4GHz
    and resets on stalls — filler matmuls bridge the phase-1 -> phase-2
    boundary.
  * xe and ee interleave per-slot in one xee tensor so one DMA stream
    feeds both sides of the phase-1 matmuls in consumption order.

Sharding: 204 north DFT chunks -> 8 cores x 27 slots as 12 ring-pair
groups per core, sizes [4,3,3,3,3,2,2,2,2,1,1,1]; each slot carries the
mirrored south ring's chunk (same E). Host sums the 8 partial outputs.
"""
import numpy as np

NLAT, LMAX, MMAX = 192, 128, 128
B, V = 2, 64
BF = B * V
NCORES = 8
CHUNK = 128
GSIZES = [4, 3, 3, 3, 3, 2, 2, 2, 2, 1, 1, 1]
NG = len(GSIZES)                  # 12 ring-pair groups per core
NSLOT = sum(GSIZES)               # 27
NROWS = 2 * NG                    # 24 rings per core
GOFF = np.concatenate([[0], np.cumsum(GSIZES)]).astype(np.int64)
MAX_NLON = 400
NPTS = 40320

# exact triangle: m-th block has L = 128-m columns (l = m..127)
LLEN = [LMAX - m for m in range(MMAX)]
OFF = np.concatenate([[0], np.cumsum(LLEN)]).astype(np.int64)
PWW = int(OFF[-1])                # 8256
OUTW = 2 * PWW                    # 16512 (re & im per m)

# xee load batches (slot_start, n_slots), group-aligned; alternate
# sync/gpsimd queues in order
XEEB = [(0, 1), (1, 1), (2, 2), (4, 3), (7, 6), (13, 5), (18, 4), (22, 5)]
assert sum(n for _, n in XEEB) == NSLOT

# bounce write spans (first_group, n_groups); last span is one group so
# the final write+read chain gating phase 2 is short
SPANS = [(0, 4), (4, 4), (8, 2), (10, 2)]

N_FILLER = 30


def _octa_nlon():
    half = NLAT // 2
    north = np.array([4 * (i + 1) + 16 for i in range(half)], dtype=np.int64)
    return np.concatenate([north, north[::-1]])


def _assign_groups():
    nlon = _octa_nlon()
    nch = np.ceil(nlon[:96] / CHUNK).astype(int)
    cls = {c: sorted(np.where(nch == c)[0].tolist()) for c in (1, 2, 3, 4)}
    assert [len(cls[c]) for c in (1, 2, 3, 4)] == [28, 32, 32, 4]
    c1, c2, c3, c4 = cls[1][:], cls[2][:], cls[3][:], cls[4][:]
    cores = []
    for c in range(NCORES):
        g4 = c4.pop() if c < 4 else c2.pop()
        g3s = [c3.pop() for _ in range(4)]
        g2s = [c2.pop() for _ in range(4)] if c < 4 else \
              [c2.pop() for _ in range(3)] + [c1.pop()]
        g1s = [c1.pop() for _ in range(3)]
        cores.append([g4] + g3s + g2s + g1s)
    assert not c1 and not c2 and not c3 and not c4
    return cores, nlon


def _build_core(rings, nlon, offs, x, E_re, E_im, PwT):
    """xee: [128 j, slot, (xe 2*BF | ee 2*MMAX)] fp16; pw4: [128, PWW]."""
    xee = np.zeros((CHUNK, NSLOT, 512), np.float16)
    pw = np.zeros((NROWS, PWW), np.float16)
    for g in range(NG):
        r = rings[g]
        rs = NLAT - 1 - r
        nl = int(nlon[r])
        for t in range(GSIZES[g]):
            j0 = t * CHUNK
            if j0 >= nl:
                continue
            s = int(GOFF[g]) + t
            jlen = min(CHUNK, nl - j0)
            xee[:jlen, s, 0:BF] = x[:, offs[r] + j0: offs[r] + j0 + jlen].T
            xee[:jlen, s, BF:2 * BF] = \
                x[:, offs[rs] + j0: offs[rs] + j0 + jlen].T
            elen = min(CHUNK, MAX_NLON - j0)
            xee[:elen, s, 256:256 + MMAX] = E_re[r, j0:j0 + elen, :]
            xee[:elen, s, 256 + MMAX:] = E_im[r, j0:j0 + elen, :]
        for ns, rr in ((0, r), (1, rs)):
            row = 2 * g + ns
            for m in range(MMAX):
                pw[row, int(OFF[m]):int(OFF[m + 1])] = PwT[m, rr, m:]
    return np.ascontiguousarray(xee.reshape(CHUNK, NSLOT * 512)), pw


def _build_bass():
    import concourse.mybir as mybir
    from concourse import bacc, tile

    dt = mybir.dt
    nc = bacc.Bacc()

    xee_d = nc.dram_tensor("xee", [CHUNK, NSLOT * 512], dt.float16,
                           kind="ExternalInput")
    pw_d = nc.dram_tensor("pw", [NROWS, PWW], dt.float16,
                          kind="ExternalInput")
    outp_d = nc.dram_tensor("outp", [128, OUTW], dt.float16,
                            kind="ExternalOutput")
    # ring-major bounce: row 2g+ns, col m*256 + h*128 + bev
    gdram = nc.dram_tensor("gdram", [NROWS, MMAX * 256], dt.float16)

    with tile.TileContext(nc) as tc:
        with (
            tc.tile_pool(name="inp", bufs=1) as in_pool,
            tc.tile_pool(name="gsl", bufs=3) as gsl_pool,
            tc.tile_pool(name="osb", bufs=4) as osb_pool,
            tc.tile_pool(name="ps1a", bufs=2, space="PSUM") as ps1a,
            tc.tile_pool(name="ps1b", bufs=2, space="PSUM") as ps1b,
            tc.tile_pool(name="ps2", bufs=4, space="PSUM") as ps2,
        ):
            xee = in_pool.tile([CHUNK, NSLOT * 512], dt.float16, tag="xee")
            # pw replicated to base partitions 0/32/64 (matmul base rule);
            # quarter 3 shares base 64 via a second gbuf column range
            pw4 = in_pool.tile([96, PWW], dt.float16, tag="pw4")
            gbuf = in_pool.tile([96, 2 * 32 * 256], dt.float16, tag="gbuf")

            # --- input loads: alternate sync/scalar so gpsimd stays an
            # empty queue for the bounce writes; compact pw on sync ---
            for i, (s0, n) in enumerate(XEEB):
                c0, c1 = s0 * 512, (s0 + n) * 512
                q = (nc.sync, nc.scalar)[i % 2]
                q.dma_start(out=xee[:, c0:c1], in_=xee_d[:, c0:c1])
            nc.sync.dma_start(out=pw4[0:NROWS, :], in_=pw_d[:])

            def xs(s):          # xe part of slot s (rhs)
                return xee[:, s * 512:s * 512 + 256]

            def es(s, h):       # ee part of slot s, h=0 re / h=1 im (lhsT)
                o = s * 512 + 256 + h * 128
                return xee[:, o:o + 128]

            def cp1(idx, out, in_):
                if idx % 2 == 0:
                    nc.scalar.copy(out, in_)
                else:
                    nc.vector.tensor_copy(out, in_)

            span_of = {}
            for si, (g0, ng) in enumerate(SPANS):
                for g in range(g0, g0 + ng):
                    span_of[g] = si

            # --- phase 1: groups; span writes on scalar; q-stacked plain
            # reads on sync/gpsimd after the right span ---
            ci = 0
            gsl_cur = None
            for g in range(NG):
                si = span_of[g]
                g0, ng = SPANS[si]
                sz = GSIZES[g]
                gre = ps1a.tile([MMAX, 512], dt.float32, tag="gre")
                gim = ps1b.tile([MMAX, 512], dt.float32, tag="gim")
                for t in range(sz):
                    s = int(GOFF[g]) + t
                    st, sp = (t == 0), (t == sz - 1)
                    nc.tensor.matmul(gre[:, 0:256], es(s, 0), xs(s),
                                     start=st, stop=sp)
                    nc.tensor.matmul(gim[:, 0:256], es(s, 1), xs(s),
                                     start=st, stop=sp)
                if g == g0:
                    gsl_cur = gsl_pool.tile([MMAX, 512 * ng], dt.float16,
                                            tag=f"gsl{ng}", name="gsl")
                go = 512 * (g - g0)
                cp1(ci + 0, gsl_cur[:, go + 0:go + 128], gre[:, 0:128])
                cp1(ci + 1, gsl_cur[:, go + 128:go + 256], gim[:, 0:128])
                cp1(ci + 2, gsl_cur[:, go + 256:go + 384], gre[:, 128:256])
                cp1(ci + 3, gsl_cur[:, go + 384:go + 512], gim[:, 128:256])
                ci += 4
                if g == 9:
                    # pw replicas to bases 32/64 on scalar's queue (free
                    # after its loads); needed when phase 2 reaches m=32
                    nc.scalar.dma_start(out=pw4[32:32 + NROWS, :],
                                        in_=pw4[0:NROWS, :])
                    nc.scalar.dma_start(out=pw4[64:64 + NROWS, :],
                                        in_=pw4[0:NROWS, :])
                if g == g0 + ng - 1:
                    # span write into ring-major rows (src 128 SBUF rows,
                    # 512B descriptors); sync has no compute duties
                    wdst = gdram[2 * g0:2 * (g0 + ng)].rearrange(
                        "k (m e) -> m k e", m=MMAX)
                    wsrc = gsl_cur[:, :].rearrange(
                        "m (k e) -> m k e", k=2 * ng)
                    nc.gpsimd.dma_start(out=wdst, in_=wsrc)
                    # q-stacked plain full reads after the last span; q0
                    # first — phase 2's first m-block only needs q0
                    if si == len(SPANS) - 1:
                        rqs = (nc.sync, nc.gpsimd, nc.sync, nc.gpsimd)
                        for q4 in range(4):
                            rb = 32 * q4 if q4 < 3 else 64
                            cb = 0 if q4 < 3 else 8192
                            rqs[q4].dma_start(
                                out=gbuf[rb:rb + NROWS, cb:cb + 8192],
                                in_=gdram[:, q4 * 8192:(q4 + 1) * 8192])

            # --- fillers: keep the PE p-state ramp alive across the
            # phase-1 -> phase-2 boundary (psum scratch, never read) ---
            for f in range(N_FILLER):
                pool, tg = (ps1a, "gre") if f % 2 == 0 else (ps1b, "gim")
                fil = pool.tile([MMAX, 512], dt.float32, tag=tg, name="fil")
                nc.tensor.matmul(fil[:, 0:512], es(0, 0), xee[:, 0:512],
                                 start=True, stop=True)

            # --- phase 2 ---
            oq = (nc.scalar, nc.sync, nc.scalar, nc.sync, nc.scalar)
            state = {"osb": None, "osb_fill": 0, "osb_base": 0, "banks": 0,
                     "po": None, "po_fill": 0, "oi": 0, "col": 0, "ci": ci}

            def cp2(idx, out, in_):
                if idx % 2 == 0:
                    nc.vector.tensor_copy(out, in_)
                else:
                    nc.scalar.copy(out, in_)

            def flush_bank():
                if state["po"] is None or state["po_fill"] == 0:
                    return
                f = state["osb_fill"]
                cp2(state["ci"], state["osb"][:, f:f + state["po_fill"]],
                    state["po"][:, 0:state["po_fill"]])
                state["ci"] += 1
                state["osb_fill"] = f + state["po_fill"]
                state["banks"] += 1
                state["po"] = None
                state["po_fill"] = 0

            def flush_osb():
                if state["osb"] is None or state["osb_fill"] == 0:
                    return
                oq[state["oi"] % 5].dma_start(
                    out=outp_d[:, state["osb_base"]:
                               state["osb_base"] + state["osb_fill"]],
                    in_=state["osb"][:, 0:state["osb_fill"]])
                state["oi"] += 1
                state["osb"] = None
                state["osb_fill"] = 0
                state["banks"] = 0

            for m in range(MMAX):
                L = LLEN[m]
                q4, ml = m // 32, m % 32
                rb = 32 * q4 if q4 < 3 else 64
                cb = 0 if q4 < 3 else 8192
                rhs = pw4[rb:rb + NROWS, int(OFF[m]):int(OFF[m]) + L]
                if ml == 0 and m > 0:
                    # matmuls with different base partitions cannot share a
                    # psum bank — break banks at quarter boundaries
                    flush_bank()
                    if state["banks"] == 8:
                        flush_osb()
                for h in range(2):
                    if state["po"] is not None and state["po_fill"] + L > 512:
                        flush_bank()
                        if state["banks"] == 8:
                            flush_osb()
                    if state["po"] is None:
                        if state["osb"] is None:
                            state["osb"] = osb_pool.tile(
                                [128, 4096], dt.float16, tag="osb",
                                name="osb")
                            state["osb_base"] = state["col"]
                        bi = state.get("bi", 0)
                        pool, tg = ((ps2, "po"), (ps2, "po"), (ps2, "po"),
                                    (ps2, "po"), (ps1a, "gre"),
                                    (ps1b, "gim"))[bi % 6]
                        state["bi"] = bi + 1
                        state["po"] = pool.tile([128, 512], dt.float32,
                                                tag=tg, name="po")
                    c0 = cb + ml * 256 + h * 128
                    lhsT = gbuf[rb:rb + NROWS, c0:c0 + 128]
                    nc.tensor.matmul(
                        state["po"][:, state["po_fill"]:
                                    state["po_fill"] + L],
                        lhsT, rhs, start=True, stop=True)
                    state["po_fill"] += L
                    state["col"] += L
            flush_bank()
            flush_osb()

    nc.compile()
    return nc


_CACHE = {}


def _get_compiled():
    if "nc" not in _CACHE:
        _CACHE["nc"] = _build_bass()
    return _CACHE["nc"]


def kernel(data, Pw, E_re, E_im, pad_idx):
    from concourse import bass_utils

    data = np.asarray(data)
    Pw = np.asarray(Pw, dtype=np.float32)
    E_re = np.asarray(E_re, dtype=np.float32)
    E_im = np.asarray(E_im, dtype=np.float32)

    cores, nlon = _assign_groups()
    offs = np.concatenate([[0], np.cumsum(nlon)[:-1]])
    x = np.ascontiguousarray(
        np.transpose(data, (0, 1, 3, 2)).reshape(BF, NPTS).astype(np.float32))
    PwT = np.ascontiguousarray(np.transpose(Pw, (1, 2, 0)))  # [m, n, l]

    in_maps = []
    for c in range(NCORES):
        xee, pw = _build_core(cores[c], nlon, offs, x, E_re, E_im, PwT)
        in_maps.append({"xee": xee, "pw": pw})

    nc = _get_compiled()
    res = bass_utils.run_bass_kernel_spmd(nc, in_maps, list(range(NCORES)))
    _CACHE["last_results"] = res

    total = np.zeros((BF, OUTW), np.float64)
    for r in res.results:
        total += r["outp"].astype(np.float64)
    total = total.astype(np.float32)

    cc = np.zeros((LMAX, MMAX, BF), np.complex64)
    for m in range(MMAX):
        L = LLEN[m]
        o = 2 * int(OFF[m])
        re = total[:, o:o + L]
        im = total[:, o + L:o + 2 * L]
        cc[m:, m, :] = (re + 1j * im).T
    cc = cc.reshape(LMAX, MMAX, B, V)
    out = np.transpose(cc, (2, 0, 1, 3))[:, None]
    return out.astype(np.complex64)
